# revision 41
# baseline (speedup 1.0000x reference)
"""Trainium2 Bass kernel for a 2-layer GAT (DGL-style) on a random graph.

Design (8 NeuronCores, SPMD, dst-node sharding):
  - 392 blocks of 128 dst nodes, LPT-balanced 49 blocks/core, positions
    sorted by edge count.
  - Two half-tables per layer (A = blocks 0..24, B = 25..48 of every core),
    each < 32768 rows so every dma_gather uses int16 indices directly.
  - Phase 1 (bf16 matmuls, x4-block batched DMAs on two HWDGE queues):
    feat1|el1|er1 = x @ [W1|Vl1|Vr1].  feat1 is stored fp8(e4m3) in (d,h)
    transposed column order; el1 rides in the same 512B row as bf16.
    Chunked AllGathers (A issued mid-phase, B at the end) build the tables.
  - L1 edge phase per dst block: 2 dma_gathers (tables A/B, 512B rows);
    A-gathers of the first 8 blocks are emitted early so they overlap the
    AllGather-B wall.  Indicator matrices ind/indT are precomputed on the
    host as fp8 0/1 and DMA'd on the sync HWDGE queue (no DVE is_equal).
    er per edge via per-chunk indT^T @ er1_sb matmuls; w =
    exp(leakyrelu(el+er)); rhs = [w*feat | w]; psum += ind^T @ rhs per
    chunk; normalize, bias, relu; feat2|el2|er2 = h @ Wcat2 via PE
    transpose; bf16 rows to T2_localA/B (256B rows).
  - L2 edge phase: same structure, 1 head, C=40, rhs-side weighting
    rhs2 = [w2*feat2 | w2] with plain fp8 ind as lhsT.
  - Gather/indicator pools are 8/6 deep so gathers run ahead of consumers;
    A-gathers of the first 8 blocks prefetch under the AllGather-B wall.
  - Host assembles per-core [6272,40] outputs via the block permutation.

All graph-structure data (indices, indicators, chunk counts) is precomputed
on the host at kernel() time and baked into inputs / the program.
"""

import sys
sys.path.insert(0, '/opt/trn_rl_repo')

import os
import numpy as np
import ml_dtypes

N_NODES = 50000
N_EDGES = 800000
F_IN = 256
H1, HD = 8, 32
C2 = 40
NEG_SLOPE = 0.2
NCORES = 8
P = 128
BLOCKS_PER_CORE = 49
NODES_PER_CORE = BLOCKS_PER_CORE * P      # 6272
NPAD = NCORES * NODES_PER_CORE            # 50176
NBLOCKS = NPAD // P                       # 392
ABLOCKS = 25                              # blocks in half A
AROWS = ABLOCKS * P                       # 3200 per core
BROWS = NODES_PER_CORE - AROWS            # 3072 per core
ATOT = NCORES * AROWS                     # 25600  (< 32768)
BTOT = NCORES * BROWS                     # 24576  (< 32768)
F1R = 512                                 # L1 table row BYTES (fp8 feat + bf16 el)
F2R = 128                                 # L2 table row cols (256B)
# phase-1 DMA batching groups (block start, count); A half then B half
P1_GROUPS = [(0, 4), (4, 4), (8, 4), (12, 4), (16, 4), (20, 4), (24, 1),
             (25, 4), (29, 4), (33, 4), (37, 4), (41, 4), (45, 4)]

# (d,h) permutation: new col d*H1+h  <- old col h*HD+d
_PERM_DH = np.arange(F_IN).reshape(H1, HD).T.reshape(-1)   # len 256


def _prep_graph(src, dst):
    """Block assignment, node->table-row map, per-position chunk layout."""
    src = src.astype(np.int64)
    dst = dst.astype(np.int64)
    blk_of_edge = dst // P
    blk_counts = np.bincount(blk_of_edge, minlength=NBLOCKS)

    # LPT: assign blocks to cores, 49 each, balancing edge totals
    order = np.argsort(-blk_counts)
    core_of_blk = np.zeros(NBLOCKS, np.int64)
    loads = np.zeros(NCORES, np.int64)
    fills = np.zeros(NCORES, np.int64)
    for b in order:
        cands = np.where(fills < BLOCKS_PER_CORE)[0]
        c = cands[np.argmin(loads[cands])]
        core_of_blk[b] = c
        loads[c] += blk_counts[b]
        fills[c] += 1

    # per-core position: sort own blocks by count desc
    pos_of_blk = np.zeros(NBLOCKS, np.int64)
    blocks_at = np.zeros((NCORES, BLOCKS_PER_CORE), np.int64)
    for c in range(NCORES):
        mine = np.where(core_of_blk == c)[0]
        mine = mine[np.argsort(-blk_counts[mine])]
        blocks_at[c] = mine
        pos_of_blk[mine] = np.arange(BLOCKS_PER_CORE)

    # node -> (half, table row)
    node_ids = np.arange(NPAD)
    nb = node_ids // P
    local_i = pos_of_blk[nb] * P + (node_ids % P)          # 0..6271
    ncore = core_of_blk[nb]
    in_a = local_i < AROWS
    row = np.where(in_a, ncore * AROWS + local_i,
                   ncore * BROWS + (local_i - AROWS))

    src_in_a = in_a[src]
    src_row = row[src]
    dstloc = (dst % P).astype(np.int64)

    # group edges by block
    e_order = np.argsort(blk_of_edge, kind='stable')
    blk_starts = np.zeros(NBLOCKS + 1, np.int64)
    np.cumsum(blk_counts, out=blk_starts[1:])

    # per block: A-edges then B-edges; chunk counts
    n_a = np.zeros((NCORES, BLOCKS_PER_CORE), np.int64)
    n_b = np.zeros((NCORES, BLOCKS_PER_CORE), np.int64)
    blk_a = {}
    blk_b = {}
    for b in range(NBLOCKS):
        es = e_order[blk_starts[b]:blk_starts[b + 1]]
        a = es[src_in_a[es]]
        bb = es[~src_in_a[es]]
        blk_a[b] = a
        blk_b[b] = bb
        c, i = core_of_blk[b], pos_of_blk[b]
        n_a[c, i] = -(-len(a) // P) if len(a) else 0
        n_b[c, i] = -(-len(bb) // P) if len(bb) else 0
    n_a_max = n_a.max(axis=0)
    n_b_max = n_b.max(axis=0)
    nb_tot = n_a_max + n_b_max
    return dict(core_of_blk=core_of_blk, pos_of_blk=pos_of_blk,
                blocks_at=blocks_at, blk_a=blk_a, blk_b=blk_b,
                src_row=src_row, dstloc=dstloc,
                n_a_max=n_a_max, n_b_max=n_b_max, nb_tot=nb_tot)


def _pack_idx16(vals):
    """dma_gather index layout: [128, n/16], wrapped by 16, replicated x8."""
    v = np.asarray(vals, np.uint16).reshape(-1, 16).T      # [16, n/16]
    return np.tile(v, (8, 1)).view(np.int16)               # [128, n/16]


def _build_core_inputs(g, features, W1, attn_l1, attn_r1, W2, attn_l2,
                       attn_r2, b1, b2):
    fp8 = ml_dtypes.float8_e4m3
    NBMAX = int(g['nb_tot'].max())
    CTOT = int(g['nb_tot'].sum())

    # Wcat1 = [W1(d,h-permuted) | Vl1 | Vr1]  (bf16)
    Vl1 = np.einsum('khd,hd->kh', W1.reshape(F_IN, H1, HD), attn_l1)
    Vr1 = np.einsum('khd,hd->kh', W1.reshape(F_IN, H1, HD), attn_r1)
    Wcat1 = np.concatenate([W1[:, _PERM_DH], Vl1, Vr1], axis=1)
    Wcat1 = Wcat1.astype(ml_dtypes.bfloat16)               # [256, 272]
    # Wcat2 rows permuted to (d,h) order to match h's layout
    vl2 = W2 @ attn_l2[0]
    vr2 = W2 @ attn_r2[0]
    Wcat2 = np.concatenate([W2, vl2[:, None], vr2[:, None]], axis=1)
    Wcat2 = Wcat2[_PERM_DH].astype(ml_dtypes.bfloat16)     # [256, 42]

    b1B = np.tile(b1[_PERM_DH].astype(np.float32)[None, :], (P, 1))
    b2B = np.tile(b2.astype(np.float32)[None, :], (P, 1))

    feats_pad = np.zeros((NPAD, F_IN), np.float32)
    feats_pad[:N_NODES] = features

    per_core = []
    for c in range(NCORES):
        my_nodes = (g['blocks_at'][c][:, None] * P
                    + np.arange(P)[None, :]).reshape(-1)
        xT = feats_pad[my_nodes].T.astype(ml_dtypes.bfloat16).copy()  # [256,6272]

        idx1_cols = []
        onehot = np.zeros((CTOT, P, P), np.uint8)   # [chunk, edge, dstloc]
        ccol = 0
        for i in range(BLOCKS_PER_CORE):
            b = g['blocks_at'][c][i]
            for kind in ('a', 'b'):
                nch = int((g['n_a_max'] if kind == 'a' else g['n_b_max'])[i])
                if nch == 0:
                    continue
                es = g['blk_a' if kind == 'a' else 'blk_b'][b]
                rows = g['src_row'][es]
                dl = g['dstloc'][es]
                npad_e = nch * P - len(es)
                rows = np.concatenate([rows, np.zeros(npad_e, np.int64)])
                idx1_cols.append(_pack_idx16(rows))
                ne = len(es)
                if ne:
                    ch_idx = ccol + np.arange(ne) // P
                    onehot[ch_idx, np.arange(ne) % P, dl] = 1
                ccol += nch
        assert ccol == CTOT
        idx_all = np.concatenate(idx1_cols, axis=1)          # [128, CTOT*8]

        # interleaved [ind | indT] per chunk: [128, CTOT, 256] fp8
        comb = np.zeros((P, CTOT, 2 * P), np.uint8)
        comb[:, :, :P] = onehot.transpose(1, 0, 2)          # ind[e,c,j]
        comb[:, :, P:] = onehot.transpose(2, 0, 1)          # indT[p,c,e]
        comb = comb.astype(fp8)

        per_core.append(dict(xT=xT, idx=idx_all, indc=comb.reshape(P, -1),
                             Wcat1=np.asarray(Wcat1), Wcat2=np.asarray(Wcat2),
                             b1B=b1B, b2B=b2B))
    return per_core, NBMAX, CTOT


def _build_program(g, NBMAX, CTOT, IDXCOLS):
    import concourse.bass as bass
    import concourse.bacc as bacc
    import concourse.mybir as mybir
    import concourse.tile as tile
    from concourse.masks import make_identity

    f32, bf16 = mybir.dt.float32, mybir.dt.bfloat16
    i16, f8 = mybir.dt.int16, mybir.dt.float8e4
    u8 = mybir.dt.uint8
    Alu, Act = mybir.AluOpType, mybir.ActivationFunctionType
    n_a, n_b = g['n_a_max'], g['n_b_max']
    nb_tot = g['nb_tot']
    NBH = NBMAX * H1

    nc = bacc.Bacc(None, target_bir_lowering=False, debug=False,
                   num_swdge_queues=4)

    t_xT = nc.dram_tensor("xT", [F_IN, NODES_PER_CORE], bf16,
                          kind="ExternalInput")
    t_idx = nc.dram_tensor("idx", [P, IDXCOLS], i16, kind="ExternalInput")
    t_ind = nc.dram_tensor("indc", [P, CTOT * 2 * P], f8, kind="ExternalInput")
    t_W1 = nc.dram_tensor("Wcat1", [F_IN, 272], bf16, kind="ExternalInput")
    t_W2 = nc.dram_tensor("Wcat2", [F_IN, 42], bf16, kind="ExternalInput")
    t_b1 = nc.dram_tensor("b1B", [P, F_IN], f32, kind="ExternalInput")
    t_b2 = nc.dram_tensor("b2B", [P, C2], f32, kind="ExternalInput")
    t_out = nc.dram_tensor("out2", [NODES_PER_CORE, C2], f32,
                           kind="ExternalOutput")

    with tile.TileContext(nc) as tc:
        with tc.tile_pool(name="dram", bufs=1, space="DRAM") as dram, \
             tc.tile_pool(name="const", bufs=1) as cst, \
             tc.tile_pool(name="resid", bufs=1) as res, \
             tc.tile_pool(name="work", bufs=3) as wk, \
             tc.tile_pool(name="gath", bufs=8) as gp, \
             tc.tile_pool(name="indp", bufs=6) as indp, \
             tc.tile_pool(name="ps_agg", bufs=2, space="PSUM") as ps_agg, \
             tc.tile_pool(name="ps_er", bufs=2, space="PSUM") as ps_er, \
             tc.tile_pool(name="ps_t", bufs=2, space="PSUM") as ps_t, \
             tc.tile_pool(name="ps_f2", bufs=2, space="PSUM") as ps_f2:

            T1_localA = dram.tile([AROWS, F1R], u8)
            T1_localB = dram.tile([BROWS, F1R], u8)
            T1A = dram.tile([ATOT, F1R], u8, addr_space="Shared")
            T1B = dram.tile([BTOT, F1R], u8, addr_space="Shared")
            T2_localA = dram.tile([AROWS, F2R], bf16)
            T2_localB = dram.tile([BROWS, F2R], bf16)
            T2A = dram.tile([ATOT, F2R], bf16, addr_space="Shared")
            T2B = dram.tile([BTOT, F2R], bf16, addr_space="Shared")

            # ---- constants ----
            b1B = cst.tile([P, F_IN], f32)
            nc.sync.dma_start(b1B[:], t_b1[:])
            b2B = cst.tile([P, C2], f32)
            nc.sync.dma_start(b2B[:], t_b2[:])
            Wc2 = cst.tile([P, 2, 42], bf16)
            nc.sync.dma_start(Wc2[:, 0, :], t_W2[0:128, :])
            nc.sync.dma_start(Wc2[:, 1, :], t_W2[128:256, :])
            ident = cst.tile([P, P], f32)
            make_identity(nc, ident[:])
            alpha = cst.tile([P, 1], f32)
            nc.vector.memset(alpha[:], NEG_SLOPE)
            er1_sb = res.tile([P, BLOCKS_PER_CORE * H1], bf16)
            er2_sb = res.tile([P, BLOCKS_PER_CORE], bf16)
            idx_sb = res.tile([P, IDXCOLS], i16)
            nc.sync.dma_start(idx_sb[:], t_idx[:])

            qctr = [0]
            offA = np.zeros(BLOCKS_PER_CORE, np.int64)
            offB = np.zeros(BLOCKS_PER_CORE, np.int64)
            _o = 0
            for _b in range(BLOCKS_PER_CORE):
                offA[_b] = _o
                offB[_b] = _o + int(n_a[_b]) * 8
                _o += int(nb_tot[_b]) * 8

            def emit_seg(Gt, tbl, n0, m, icol, elem, sp=False):
                if m <= 0:
                    return
                q = qctr[0] % 4
                qctr[0] += 1
                nc.gpsimd.dma_gather(
                    Gt[:, n0:n0 + m, :], tbl,
                    idx_sb[:, icol:icol + m * 8], m * P, m * P, elem,
                    single_packet=sp, queue_num=q)

            def allgather(t_in, t_out):
                nc.gpsimd.collective_compute(
                    "AllGather", mybir.AluOpType.bypass,
                    replica_groups=[list(range(NCORES))],
                    ins=[t_in[:]], outs=[t_out[:]])

            # ---- phase 1: feat1(d,h)|el1|er1 = x @ [W1p|Vl1|Vr1] ----
            with tc.tile_pool(name="p1", bufs=4) as p1, \
                 tc.tile_pool(name="p1w", bufs=1) as p1w:
                w1a = p1w.tile([P, 272], bf16)
                nc.sync.dma_start(w1a[:], t_W1[0:128, :])
                w1b = p1w.tile([P, 272], bf16)
                nc.sync.dma_start(w1b[:], t_W1[128:256, :])
                for (g0, gs) in P1_GROUPS:
                    sl = slice(g0 * P, (g0 + gs) * P)
                    xt = p1.tile([P, 2, 4 * P], bf16, tag="xt")
                    nc.scalar.dma_start(xt[:, 0, 0:gs * P], t_xT[0:128, sl])
                    nc.sync.dma_start(xt[:, 1, 0:gs * P], t_xT[128:256, sl])
                    fbg8 = p1.tile([P, 4, 256], u8, tag="p1out8")
                    fbgel = p1.tile([P, 4, 16], u8, tag="p1oute")
                    for j in range(gs):
                        b = g0 + j
                        acc = ps_t.tile([P, 272], f32, space="PSUM", tag="htp")
                        nc.tensor.matmul(acc[:], lhsT=xt[:, 0, j * P:(j + 1) * P],
                                         rhs=w1a[:], start=True, stop=False)
                        nc.tensor.matmul(acc[:], lhsT=xt[:, 1, j * P:(j + 1) * P],
                                         rhs=w1b[:], start=False, stop=True)
                        nc.vector.tensor_copy(out=fbg8[:, j, :].bitcast(f8),
                                              in_=acc[:, 0:256])
                        nc.vector.tensor_copy(out=fbgel[:, j, :].bitcast(bf16),
                                              in_=acc[:, 256:264])
                        nc.vector.tensor_copy(
                            out=er1_sb[:, b * H1:(b + 1) * H1],
                            in_=acc[:, 264:272])
                    if g0 < ABLOCKS:
                        oap8, oape = T1_localA[sl, 0:256], T1_localA[sl, 256:272]
                    else:
                        sb = slice((g0 - ABLOCKS) * P, (g0 - ABLOCKS + gs) * P)
                        oap8, oape = T1_localB[sb, 0:256], T1_localB[sb, 256:272]
                    nc.sync.dma_start(
                        oap8.rearrange("(j p) c -> p j c", p=P),
                        fbg8[:, 0:gs, :])
                    nc.scalar.dma_start(
                        oape.rearrange("(j p) c -> p j c", p=P),
                        fbgel[:, 0:gs, :])
                    if g0 + gs == ABLOCKS:
                        allgather(T1_localA, T1A)
            allgather(T1_localB, T1B)

            # ---- layer 1 edge phase ----
            NPREF = 8
            ccol = 0
            g1_pref = []
            for b in range(NPREF):
                Gp = gp.tile([P, NBMAX, F1R], u8, tag="g1")
                emit_seg(Gp, T1A[:], 0, int(n_a[b]), int(offA[b]), F1R)
                g1_pref.append(Gp)
            for b in range(BLOCKS_PER_CORE):
                nbi = int(nb_tot[b])
                na, nbk = int(n_a[b]), int(n_b[b])
                if b < NPREF:
                    G = g1_pref[b]
                else:
                    G = gp.tile([P, NBMAX, F1R], u8, tag="g1")
                    emit_seg(G, T1A[:], 0, na, int(offA[b]), F1R)
                emit_seg(G, T1B[:], na, nbk, int(offB[b]), F1R)

                ic = indp.tile([P, NBMAX, 2 * P], f8, tag="ind")
                nc.sync.dma_start(ic[:, 0:nbi, :],
                                    t_ind[:, ccol * 2 * P:(ccol + nbi) * 2 * P])

                ers = ps_er.tile([P, NBH], f32, space="PSUM", tag="ers")
                for c in range(nbi):
                    nc.tensor.matmul(ers[:, c * H1:(c + 1) * H1],
                                     lhsT=ic[:, c, P:2 * P],
                                     rhs=er1_sb[:, b * H1:(b + 1) * H1],
                                     start=True, stop=True)
                ee = wk.tile([P, NBMAX * H1], f32, tag="ee")
                nc.vector.tensor_tensor(
                    out=ee[:, 0:nbi * H1].rearrange("p (a h) -> p a h", h=H1),
                    in0=G[:, 0:nbi, 256:272].bitcast(bf16),
                    in1=ers[:, 0:nbi * H1].rearrange("p (a h) -> p a h", h=H1),
                    op=Alu.add)
                nc.scalar.activation(ee[:, 0:nbi * H1], ee[:, 0:nbi * H1],
                                     Act.Prelu, alpha=alpha[:, :1])
                w = wk.tile([P, NBMAX * H1], bf16, tag="w")
                nc.scalar.activation(w[:, 0:nbi * H1], ee[:, 0:nbi * H1],
                                     Act.Exp)
                rhs_all = wk.tile([P, NBMAX, 264], bf16, tag="rhsall")
                nc.scalar.copy(
                    out=rhs_all[:, 0:nbi, F_IN:264],
                    in_=w[:, 0:nbi * H1].rearrange("p (a h) -> p a h", h=H1))
                # (d,h) layout: inner dim h unit-stride on all operands -> 2x
                nc.vector.tensor_tensor(
                    out=rhs_all[:, 0:nbi, 0:F_IN]
                        .rearrange("p a (d h) -> p a d h", h=H1),
                    in0=G[:, 0:nbi, 0:256].bitcast(f8)
                        .rearrange("p a (d h) -> p a d h", h=H1),
                    in1=w[:, 0:nbi * H1]
                        .rearrange("p (a h) -> p a h", h=H1)[:, :, None, :]
                        .to_broadcast([P, nbi, HD, H1]),
                    op=Alu.mult)
                acc = ps_agg.tile([P, 264], f32, space="PSUM", tag="agg")
                for c in range(nbi):
                    nc.tensor.matmul(acc[:], lhsT=ic[:, c, 0:P],
                                     rhs=rhs_all[:, c, :],
                                     start=(c == 0), stop=(c == nbi - 1))

                den = wk.tile([P, H1], f32, tag="den")
                nc.vector.tensor_scalar_max(den[:], acc[:, F_IN:264], 1e-30)
                rec = wk.tile([P, H1], f32, tag="rec")
                nc.vector.reciprocal(rec[:], den[:])
                h = wk.tile([P, F_IN], f32, tag="h")
                nc.vector.tensor_tensor(
                    out=h[:].rearrange("p (d h) -> p d h", h=H1),
                    in0=acc[:, 0:F_IN].rearrange("p (d h) -> p d h", h=H1),
                    in1=rec[:, None, :].to_broadcast([P, HD, H1]),
                    op=Alu.mult)
                hb = wk.tile([P, F_IN], f32, tag="hb")
                nc.vector.tensor_tensor(out=hb[:], in0=h[:], in1=b1B[:],
                                        op=Alu.add)
                nc.scalar.activation(hb[:], hb[:], Act.Relu)

                f2 = ps_f2.tile([P, 42], f32, space="PSUM", tag="f2")
                for j in range(2):
                    ht_ps = ps_t.tile([P, 272], f32, space="PSUM", tag="htp")
                    nc.tensor.transpose(ht_ps[:, 0:P], hb[:, j * P:(j + 1) * P],
                                        ident[:])
                    ht = wk.tile([P, P], bf16, tag="ht")
                    nc.scalar.copy(out=ht[:], in_=ht_ps[:, 0:P])
                    nc.tensor.matmul(f2[:], lhsT=ht[:], rhs=Wc2[:, j, :],
                                     start=(j == 0), stop=(j == 1))
                t2r = wk.tile([P, 41], bf16, tag="t2r")
                nc.scalar.copy(out=t2r[:], in_=f2[:, 0:41])
                nc.vector.tensor_copy(out=er2_sb[:, b:b + 1], in_=f2[:, 41:42])
                sl = slice(b * P, (b + 1) * P)
                if b < ABLOCKS:
                    oap = T2_localA[sl, 0:41]
                else:
                    oap = T2_localB[(b - ABLOCKS) * P:(b - ABLOCKS + 1) * P, 0:41]
                nc.sync.dma_start(oap, t2r[:])
                ccol += nbi
                if b == ABLOCKS - 1:
                    allgather(T2_localA, T2A)
            allgather(T2_localB, T2B)

            # ---- layer 2 edge phase ----
            ccol = 0
            g2_pref = []
            for b in range(NPREF):
                Gp = gp.tile([P, NBMAX, F2R], bf16, tag="g2")
                emit_seg(Gp, T2A[:], 0, int(n_a[b]), int(offA[b]), F2R)
                g2_pref.append(Gp)
            for b in range(BLOCKS_PER_CORE):
                nbi = int(nb_tot[b])
                na, nbk = int(n_a[b]), int(n_b[b])
                if b < NPREF:
                    G2 = g2_pref[b]
                else:
                    G2 = gp.tile([P, NBMAX, F2R], bf16, tag="g2")
                    emit_seg(G2, T2A[:], 0, na, int(offA[b]), F2R)
                emit_seg(G2, T2B[:], na, nbk, int(offB[b]), F2R)

                ic = indp.tile([P, NBMAX, 2 * P], f8, tag="ind")
                nc.sync.dma_start(ic[:, 0:nbi, :],
                                    t_ind[:, ccol * 2 * P:(ccol + nbi) * 2 * P])

                ers = ps_er.tile([P, NBH], f32, space="PSUM", tag="ers")
                for c in range(nbi):
                    nc.tensor.matmul(ers[:, c:c + 1],
                                     lhsT=ic[:, c, P:2 * P],
                                     rhs=er2_sb[:, b:b + 1],
                                     start=True, stop=True)
                ee = wk.tile([P, NBMAX], f32, tag="ee2")
                nc.vector.tensor_tensor(
                    out=ee[:, 0:nbi],
                    in0=G2[:, 0:nbi, C2:C2 + 1].rearrange("p a b -> p (a b)"),
                    in1=ers[:, 0:nbi], op=Alu.add)
                nc.scalar.activation(ee[:, 0:nbi], ee[:, 0:nbi], Act.Prelu,
                                     alpha=alpha[:, :1])
                w2 = wk.tile([P, NBMAX], bf16, tag="w2")
                nc.scalar.activation(w2[:, 0:nbi], ee[:, 0:nbi], Act.Exp)

                rhs2 = wk.tile([P, NBMAX, 41], bf16, tag="rhs2a")
                nc.vector.tensor_tensor(
                    out=rhs2[:, 0:nbi, 0:C2],
                    in0=G2[:, 0:nbi, 0:C2],
                    in1=w2[:, 0:nbi, None].to_broadcast([P, nbi, C2]),
                    op=Alu.mult)
                nc.scalar.copy(out=rhs2[:, 0:nbi, C2:41],
                               in_=w2[:, 0:nbi, None])
                acc = ps_agg.tile([P, 264], f32, space="PSUM", tag="agg")
                for c in range(nbi):
                    nc.tensor.matmul(acc[:, 0:41], lhsT=ic[:, c, 0:P],
                                     rhs=rhs2[:, c, :],
                                     start=(c == 0), stop=(c == nbi - 1))

                den = wk.tile([P, 1], f32, tag="den2")
                nc.vector.tensor_scalar_max(den[:], acc[:, C2:41], 1e-30)
                rec = wk.tile([P, 1], f32, tag="rec2")
                nc.vector.reciprocal(rec[:], den[:])
                o = wk.tile([P, C2], f32, tag="o")
                nc.vector.tensor_tensor(
                    out=o[:], in0=acc[:, 0:C2],
                    in1=rec[:, :1].to_broadcast([P, C2]), op=Alu.mult)
                nc.vector.tensor_tensor(out=o[:], in0=o[:], in1=b2B[:],
                                        op=Alu.add)
                nc.scalar.dma_start(t_out[b * P:(b + 1) * P, :], o[:])
                ccol += nbi

    nc.compile()
    return nc


def kernel(features, src, dst, W1, attn_l1, attn_r1, b1, W2, attn_l2,
           attn_r2, b2):
    from concourse import bass_utils

    features = np.asarray(features, np.float32)
    src = np.asarray(src)
    dst = np.asarray(dst)
    W1 = np.asarray(W1, np.float32)
    attn_l1 = np.asarray(attn_l1, np.float32)
    attn_r1 = np.asarray(attn_r1, np.float32)
    b1 = np.asarray(b1, np.float32)
    W2 = np.asarray(W2, np.float32)
    attn_l2 = np.asarray(attn_l2, np.float32)
    attn_r2 = np.asarray(attn_r2, np.float32)
    b2 = np.asarray(b2, np.float32)

    g = _prep_graph(src, dst)
    per_core, NBMAX, CTOT = _build_core_inputs(
        g, features, W1, attn_l1, attn_r1, W2, attn_l2, attn_r2, b1, b2)

    IDXCOLS = per_core[0]['idx'].shape[1]
    nc = _build_program(g, NBMAX, CTOT, IDXCOLS)

    in_maps = []
    for pc in per_core:
        in_maps.append({
            "xT": pc['xT'], "idx": pc['idx'], "indc": pc['indc'],
            "Wcat1": pc['Wcat1'], "Wcat2": pc['Wcat2'],
            "b1B": pc['b1B'], "b2B": pc['b2B'],
        })

    res = bass_utils.run_bass_kernel_spmd(
        nc, in_maps, core_ids=list(range(NCORES)),
        trace=bool(int(os.environ.get('KTRACE', '0'))))
    kernel.last_result = res

    out = np.zeros((N_NODES, C2), np.float32)
    for c in range(NCORES):
        oc = res.results[c]["out2"]
        for i in range(BLOCKS_PER_CORE):
            b = g['blocks_at'][c][i]
            lo = b * P
            hi = min(lo + P, N_NODES)
            if hi > lo:
                out[lo:hi] = oc[i * P: i * P + (hi - lo)]
    return out


kernel.last_result = None


# revision 42
# speedup vs baseline: 1.0091x; 1.0091x over previous
"""Trainium2 Bass kernel for a 2-layer GAT (DGL-style) on a random graph.

Design (8 NeuronCores, SPMD, dst-node sharding):
  - 392 blocks of 128 dst nodes, LPT-balanced 49 blocks/core, positions
    sorted by edge count.
  - Two half-tables per layer (A = blocks 0..24, B = 25..48 of every core),
    each < 32768 rows so every dma_gather uses int16 indices directly.
  - Phase 1 (bf16 matmuls, x4-block batched DMAs on two HWDGE queues):
    feat1|el1|er1 = x @ [W1|Vl1|Vr1].  feat1 is stored fp8(e4m3) in (d,h)
    transposed column order; el1 rides in the same 512B row as bf16.
    Chunked AllGathers (A issued mid-phase, B at the end) build the tables.
  - L1 edge phase per dst block: 2 dma_gathers (tables A/B, 512B rows);
    A-gathers of the first 8 blocks are emitted early so they overlap the
    AllGather-B wall.  Indicator matrices ind/indT are precomputed on the
    host as fp8 0/1 and DMA'd on the sync HWDGE queue (no DVE is_equal).
    er per edge via per-chunk indT^T @ er1_sb matmuls; w =
    exp(leakyrelu(el+er)); rhs = [w*feat | w]; psum += ind^T @ rhs per
    chunk; normalize, bias, relu; feat2|el2|er2 = h @ Wcat2 via PE
    transpose; bf16 rows to T2_localA/B (256B rows).
  - L2 edge phase: same structure, 1 head, C=40, rhs-side weighting
    rhs2 = [w2*feat2 | w2] with plain fp8 ind as lhsT.
  - Gather/indicator pools are 8/6 deep so gathers run ahead of consumers;
    A-gathers of the first 8 blocks prefetch under the AllGather-B wall.
  - Host assembles per-core [6272,40] outputs via the block permutation.

All graph-structure data (indices, indicators, chunk counts) is precomputed
on the host at kernel() time and baked into inputs / the program.
"""

import sys
sys.path.insert(0, '/opt/trn_rl_repo')

import os
import numpy as np
import ml_dtypes

N_NODES = 50000
N_EDGES = 800000
F_IN = 256
H1, HD = 8, 32
C2 = 40
NEG_SLOPE = 0.2
NCORES = 8
P = 128
BLOCKS_PER_CORE = 49
NODES_PER_CORE = BLOCKS_PER_CORE * P      # 6272
NPAD = NCORES * NODES_PER_CORE            # 50176
NBLOCKS = NPAD // P                       # 392
ABLOCKS = 25                              # blocks in half A
AROWS = ABLOCKS * P                       # 3200 per core
BROWS = NODES_PER_CORE - AROWS            # 3072 per core
ATOT = NCORES * AROWS                     # 25600  (< 32768)
BTOT = NCORES * BROWS                     # 24576  (< 32768)
F1R = 512                                 # L1 table row BYTES (fp8 feat + bf16 el)
F2R = 128                                 # L2 table row cols (256B)
# phase-1 DMA batching groups (block start, count); A half then B half
P1_GROUPS = [(0, 4), (4, 4), (8, 4), (12, 4), (16, 4), (20, 4), (24, 1),
             (25, 4), (29, 4), (33, 4), (37, 4), (41, 4), (45, 4)]

# (d,h) permutation: new col d*H1+h  <- old col h*HD+d
_PERM_DH = np.arange(F_IN).reshape(H1, HD).T.reshape(-1)   # len 256


def _prep_graph(src, dst):
    """Block assignment, node->table-row map, per-position chunk layout."""
    src = src.astype(np.int64)
    dst = dst.astype(np.int64)
    blk_of_edge = dst // P
    blk_counts = np.bincount(blk_of_edge, minlength=NBLOCKS)

    # LPT: assign blocks to cores, 49 each, balancing edge totals
    order = np.argsort(-blk_counts)
    core_of_blk = np.zeros(NBLOCKS, np.int64)
    loads = np.zeros(NCORES, np.int64)
    fills = np.zeros(NCORES, np.int64)
    for b in order:
        cands = np.where(fills < BLOCKS_PER_CORE)[0]
        c = cands[np.argmin(loads[cands])]
        core_of_blk[b] = c
        loads[c] += blk_counts[b]
        fills[c] += 1

    # per-core position: sort own blocks by count desc
    pos_of_blk = np.zeros(NBLOCKS, np.int64)
    blocks_at = np.zeros((NCORES, BLOCKS_PER_CORE), np.int64)
    for c in range(NCORES):
        mine = np.where(core_of_blk == c)[0]
        mine = mine[np.argsort(-blk_counts[mine])]
        blocks_at[c] = mine
        pos_of_blk[mine] = np.arange(BLOCKS_PER_CORE)

    # node -> (half, table row)
    node_ids = np.arange(NPAD)
    nb = node_ids // P
    local_i = pos_of_blk[nb] * P + (node_ids % P)          # 0..6271
    ncore = core_of_blk[nb]
    in_a = local_i < AROWS
    row = np.where(in_a, ncore * AROWS + local_i,
                   ncore * BROWS + (local_i - AROWS))

    src_in_a = in_a[src]
    src_row = row[src]
    dstloc = (dst % P).astype(np.int64)

    # group edges by block
    e_order = np.argsort(blk_of_edge, kind='stable')
    blk_starts = np.zeros(NBLOCKS + 1, np.int64)
    np.cumsum(blk_counts, out=blk_starts[1:])

    # per block: A-edges then B-edges; chunk counts
    n_a = np.zeros((NCORES, BLOCKS_PER_CORE), np.int64)
    n_b = np.zeros((NCORES, BLOCKS_PER_CORE), np.int64)
    blk_a = {}
    blk_b = {}
    for b in range(NBLOCKS):
        es = e_order[blk_starts[b]:blk_starts[b + 1]]
        a = es[src_in_a[es]]
        bb = es[~src_in_a[es]]
        blk_a[b] = a
        blk_b[b] = bb
        c, i = core_of_blk[b], pos_of_blk[b]
        n_a[c, i] = -(-len(a) // P) if len(a) else 0
        n_b[c, i] = -(-len(bb) // P) if len(bb) else 0
    n_a_max = n_a.max(axis=0)
    n_b_max = n_b.max(axis=0)
    nb_tot = n_a_max + n_b_max
    return dict(core_of_blk=core_of_blk, pos_of_blk=pos_of_blk,
                blocks_at=blocks_at, blk_a=blk_a, blk_b=blk_b,
                src_row=src_row, dstloc=dstloc,
                n_a_max=n_a_max, n_b_max=n_b_max, nb_tot=nb_tot)


def _pack_idx16(vals):
    """dma_gather index layout: [128, n/16], wrapped by 16, replicated x8."""
    v = np.asarray(vals, np.uint16).reshape(-1, 16).T      # [16, n/16]
    return np.tile(v, (8, 1)).view(np.int16)               # [128, n/16]


def _build_core_inputs(g, features, W1, attn_l1, attn_r1, W2, attn_l2,
                       attn_r2, b1, b2):
    fp8 = ml_dtypes.float8_e4m3
    NBMAX = int(g['nb_tot'].max())
    CTOT = int(g['nb_tot'].sum())

    # Wcat1 = [W1(d,h-permuted) | Vl1 | Vr1]  (bf16)
    Vl1 = np.einsum('khd,hd->kh', W1.reshape(F_IN, H1, HD), attn_l1)
    Vr1 = np.einsum('khd,hd->kh', W1.reshape(F_IN, H1, HD), attn_r1)
    Wcat1 = np.concatenate([W1[:, _PERM_DH], Vl1, Vr1], axis=1)
    Wcat1 = Wcat1.astype(ml_dtypes.bfloat16)               # [256, 272]
    # Wcat2 rows permuted to (d,h) order to match h's layout
    vl2 = W2 @ attn_l2[0]
    vr2 = W2 @ attn_r2[0]
    Wcat2 = np.concatenate([W2, vl2[:, None], vr2[:, None]], axis=1)
    Wcat2 = Wcat2[_PERM_DH].astype(ml_dtypes.bfloat16)     # [256, 42]

    b1B = np.tile(b1[_PERM_DH].astype(np.float32)[None, :], (P, 1))
    b2B = np.tile(b2.astype(np.float32)[None, :], (P, 1))

    feats_pad = np.zeros((NPAD, F_IN), np.float32)
    feats_pad[:N_NODES] = features

    per_core = []
    for c in range(NCORES):
        my_nodes = (g['blocks_at'][c][:, None] * P
                    + np.arange(P)[None, :]).reshape(-1)
        xT = feats_pad[my_nodes].T.astype(ml_dtypes.bfloat16).copy()  # [256,6272]

        idx1_cols = []
        onehot = np.zeros((CTOT, P, P), np.uint8)   # [chunk, edge, dstloc]
        ccol = 0
        for i in range(BLOCKS_PER_CORE):
            b = g['blocks_at'][c][i]
            for kind in ('a', 'b'):
                nch = int((g['n_a_max'] if kind == 'a' else g['n_b_max'])[i])
                if nch == 0:
                    continue
                es = g['blk_a' if kind == 'a' else 'blk_b'][b]
                rows = g['src_row'][es]
                dl = g['dstloc'][es]
                npad_e = nch * P - len(es)
                rows = np.concatenate([rows, np.zeros(npad_e, np.int64)])
                idx1_cols.append(_pack_idx16(rows))
                ne = len(es)
                if ne:
                    ch_idx = ccol + np.arange(ne) // P
                    onehot[ch_idx, np.arange(ne) % P, dl] = 1
                ccol += nch
        assert ccol == CTOT
        idx_all = np.concatenate(idx1_cols, axis=1)          # [128, CTOT*8]

        # interleaved [ind | indT] per chunk: [128, CTOT, 256] fp8
        comb = np.zeros((P, CTOT, 2 * P), np.uint8)
        comb[:, :, :P] = onehot.transpose(1, 0, 2)          # ind[e,c,j]
        comb[:, :, P:] = onehot.transpose(2, 0, 1)          # indT[p,c,e]
        comb = comb.astype(fp8)

        per_core.append(dict(xT=xT, idx=idx_all, indc=comb.reshape(P, -1),
                             Wcat1=np.asarray(Wcat1), Wcat2=np.asarray(Wcat2),
                             b1B=b1B, b2B=b2B))
    return per_core, NBMAX, CTOT


def _build_program(g, NBMAX, CTOT, IDXCOLS):
    import concourse.bass as bass
    import concourse.bacc as bacc
    import concourse.mybir as mybir
    import concourse.tile as tile
    from concourse.masks import make_identity

    f32, bf16 = mybir.dt.float32, mybir.dt.bfloat16
    i16, f8 = mybir.dt.int16, mybir.dt.float8e4
    u8 = mybir.dt.uint8
    Alu, Act = mybir.AluOpType, mybir.ActivationFunctionType
    n_a, n_b = g['n_a_max'], g['n_b_max']
    nb_tot = g['nb_tot']
    NBH = NBMAX * H1

    nc = bacc.Bacc(None, target_bir_lowering=False, debug=False,
                   num_swdge_queues=4)

    t_xT = nc.dram_tensor("xT", [F_IN, NODES_PER_CORE], bf16,
                          kind="ExternalInput")
    t_idx = nc.dram_tensor("idx", [P, IDXCOLS], i16, kind="ExternalInput")
    t_ind = nc.dram_tensor("indc", [P, CTOT * 2 * P], f8, kind="ExternalInput")
    t_W1 = nc.dram_tensor("Wcat1", [F_IN, 272], bf16, kind="ExternalInput")
    t_W2 = nc.dram_tensor("Wcat2", [F_IN, 42], bf16, kind="ExternalInput")
    t_b1 = nc.dram_tensor("b1B", [P, F_IN], f32, kind="ExternalInput")
    t_b2 = nc.dram_tensor("b2B", [P, C2], f32, kind="ExternalInput")
    t_out = nc.dram_tensor("out2", [NODES_PER_CORE, C2], f32,
                           kind="ExternalOutput")

    with tile.TileContext(nc) as tc:
        with tc.tile_pool(name="dram", bufs=1, space="DRAM") as dram, \
             tc.tile_pool(name="const", bufs=1) as cst, \
             tc.tile_pool(name="resid", bufs=1) as res, \
             tc.tile_pool(name="work", bufs=3) as wk, \
             tc.tile_pool(name="gath", bufs=8) as gp, \
             tc.tile_pool(name="indp", bufs=6) as indp, \
             tc.tile_pool(name="ps_agg", bufs=2, space="PSUM") as ps_agg, \
             tc.tile_pool(name="ps_er", bufs=2, space="PSUM") as ps_er, \
             tc.tile_pool(name="ps_t", bufs=2, space="PSUM") as ps_t, \
             tc.tile_pool(name="ps_f2", bufs=2, space="PSUM") as ps_f2:

            T1_localA = dram.tile([AROWS, F1R], u8)
            T1_localB = dram.tile([BROWS, F1R], u8)
            T1A = dram.tile([ATOT, F1R], u8, addr_space="Shared")
            T1B = dram.tile([BTOT, F1R], u8, addr_space="Shared")
            T2_localA = dram.tile([AROWS, F2R], bf16)
            T2_localB = dram.tile([BROWS, F2R], bf16)
            T2A = dram.tile([ATOT, F2R], bf16, addr_space="Shared")
            T2B = dram.tile([BTOT, F2R], bf16, addr_space="Shared")

            # ---- constants ----
            b1B = cst.tile([P, F_IN], f32)
            nc.sync.dma_start(b1B[:], t_b1[:])
            b2B = cst.tile([P, C2], f32)
            nc.sync.dma_start(b2B[:], t_b2[:])
            Wc2 = cst.tile([P, 2, 42], bf16)
            nc.sync.dma_start(Wc2[:, 0, :], t_W2[0:128, :])
            nc.sync.dma_start(Wc2[:, 1, :], t_W2[128:256, :])
            ident = cst.tile([P, P], f32)
            make_identity(nc, ident[:])
            alpha = cst.tile([P, 1], f32)
            nc.vector.memset(alpha[:], NEG_SLOPE)
            er1_sb = res.tile([P, BLOCKS_PER_CORE * H1], bf16)
            er2_sb = res.tile([P, BLOCKS_PER_CORE], bf16)
            idx_sb = res.tile([P, IDXCOLS], i16)
            nc.sync.dma_start(idx_sb[:], t_idx[:])

            qctr = [0]
            offA = np.zeros(BLOCKS_PER_CORE, np.int64)
            offB = np.zeros(BLOCKS_PER_CORE, np.int64)
            _o = 0
            for _b in range(BLOCKS_PER_CORE):
                offA[_b] = _o
                offB[_b] = _o + int(n_a[_b]) * 8
                _o += int(nb_tot[_b]) * 8

            def emit_seg(Gt, tbl, n0, m, icol, elem, sp=False):
                if m <= 0:
                    return
                q = qctr[0] % 4
                qctr[0] += 1
                nc.gpsimd.dma_gather(
                    Gt[:, n0:n0 + m, :], tbl,
                    idx_sb[:, icol:icol + m * 8], m * P, m * P, elem,
                    single_packet=sp, queue_num=q)

            def allgather(t_in, t_out):
                nc.gpsimd.collective_compute(
                    "AllGather", mybir.AluOpType.bypass,
                    replica_groups=[list(range(NCORES))],
                    ins=[t_in[:]], outs=[t_out[:]])

            # ---- phase 1: feat1(d,h)|el1|er1 = x @ [W1p|Vl1|Vr1] ----
            with tc.tile_pool(name="p1", bufs=4) as p1, \
                 tc.tile_pool(name="p1w", bufs=1) as p1w:
                w1a = p1w.tile([P, 272], bf16)
                nc.sync.dma_start(w1a[:], t_W1[0:128, :])
                w1b = p1w.tile([P, 272], bf16)
                nc.sync.dma_start(w1b[:], t_W1[128:256, :])
                for (g0, gs) in P1_GROUPS:
                    sl = slice(g0 * P, (g0 + gs) * P)
                    xt = p1.tile([P, 2, 4 * P], bf16, tag="xt")
                    nc.scalar.dma_start(xt[:, 0, 0:gs * P], t_xT[0:128, sl])
                    nc.sync.dma_start(xt[:, 1, 0:gs * P], t_xT[128:256, sl])
                    fbg8 = p1.tile([P, 4, 256], u8, tag="p1out8")
                    fbgel = p1.tile([P, 4, 16], u8, tag="p1oute")
                    for j in range(gs):
                        b = g0 + j
                        acc = ps_t.tile([P, 272], f32, space="PSUM", tag="htp")
                        nc.tensor.matmul(acc[:], lhsT=xt[:, 0, j * P:(j + 1) * P],
                                         rhs=w1a[:], start=True, stop=False)
                        nc.tensor.matmul(acc[:], lhsT=xt[:, 1, j * P:(j + 1) * P],
                                         rhs=w1b[:], start=False, stop=True)
                        nc.vector.tensor_copy(out=fbg8[:, j, :].bitcast(f8),
                                              in_=acc[:, 0:256])
                        nc.vector.tensor_copy(out=fbgel[:, j, :].bitcast(bf16),
                                              in_=acc[:, 256:264])
                        nc.vector.tensor_copy(
                            out=er1_sb[:, b * H1:(b + 1) * H1],
                            in_=acc[:, 264:272])
                    if g0 < ABLOCKS:
                        oap8, oape = T1_localA[sl, 0:256], T1_localA[sl, 256:272]
                    else:
                        sb = slice((g0 - ABLOCKS) * P, (g0 - ABLOCKS + gs) * P)
                        oap8, oape = T1_localB[sb, 0:256], T1_localB[sb, 256:272]
                    nc.sync.dma_start(
                        oap8.rearrange("(j p) c -> p j c", p=P),
                        fbg8[:, 0:gs, :])
                    nc.scalar.dma_start(
                        oape.rearrange("(j p) c -> p j c", p=P),
                        fbgel[:, 0:gs, :])
                    if g0 + gs == ABLOCKS:
                        allgather(T1_localA, T1A)
            allgather(T1_localB, T1B)

            # ---- layer 1 edge phase ----
            NPREF = 8
            ccol = 0
            g1_pref = []
            for b in range(NPREF):
                Gp = gp.tile([P, NBMAX, F1R], u8, tag="g1")
                emit_seg(Gp, T1A[:], 0, int(n_a[b]), int(offA[b]), F1R)
                g1_pref.append(Gp)
            for b in range(BLOCKS_PER_CORE):
                nbi = int(nb_tot[b])
                na, nbk = int(n_a[b]), int(n_b[b])
                if b < NPREF:
                    G = g1_pref[b]
                else:
                    G = gp.tile([P, NBMAX, F1R], u8, tag="g1")
                    emit_seg(G, T1A[:], 0, na, int(offA[b]), F1R)
                emit_seg(G, T1B[:], na, nbk, int(offB[b]), F1R)

                ic = indp.tile([P, NBMAX, 2 * P], f8, tag="ind")
                nc.sync.dma_start(ic[:, 0:nbi, :],
                                    t_ind[:, ccol * 2 * P:(ccol + nbi) * 2 * P])

                ers = ps_er.tile([P, NBH], f32, space="PSUM", tag="ers")
                for c in range(nbi):
                    nc.tensor.matmul(ers[:, c * H1:(c + 1) * H1],
                                     lhsT=ic[:, c, P:2 * P],
                                     rhs=er1_sb[:, b * H1:(b + 1) * H1],
                                     start=True, stop=True)
                ee = wk.tile([P, NBMAX * H1], f32, tag="ee")
                nc.vector.tensor_tensor(
                    out=ee[:, 0:nbi * H1].rearrange("p (a h) -> p a h", h=H1),
                    in0=G[:, 0:nbi, 256:272].bitcast(bf16),
                    in1=ers[:, 0:nbi * H1].rearrange("p (a h) -> p a h", h=H1),
                    op=Alu.add)
                nc.scalar.activation(ee[:, 0:nbi * H1], ee[:, 0:nbi * H1],
                                     Act.Prelu, alpha=alpha[:, :1])
                w = wk.tile([P, NBMAX * H1], bf16, tag="w")
                nc.scalar.activation(w[:, 0:nbi * H1], ee[:, 0:nbi * H1],
                                     Act.Exp)
                rhs_all = wk.tile([P, NBMAX, 264], bf16, tag="rhsall")
                nc.scalar.copy(
                    out=rhs_all[:, 0:nbi, F_IN:264],
                    in_=w[:, 0:nbi * H1].rearrange("p (a h) -> p a h", h=H1))
                # (d,h) layout: inner dim h unit-stride on all operands -> 2x
                nc.vector.tensor_tensor(
                    out=rhs_all[:, 0:nbi, 0:F_IN]
                        .rearrange("p a (d h) -> p a d h", h=H1),
                    in0=G[:, 0:nbi, 0:256].bitcast(f8)
                        .rearrange("p a (d h) -> p a d h", h=H1),
                    in1=w[:, 0:nbi * H1]
                        .rearrange("p (a h) -> p a h", h=H1)[:, :, None, :]
                        .to_broadcast([P, nbi, HD, H1]),
                    op=Alu.mult)
                acc = ps_agg.tile([P, 264], f32, space="PSUM", tag="agg")
                for c in range(nbi):
                    nc.tensor.matmul(acc[:], lhsT=ic[:, c, 0:P],
                                     rhs=rhs_all[:, c, :],
                                     start=(c == 0), stop=(c == nbi - 1))

                den = wk.tile([P, H1], f32, tag="den")
                nc.vector.tensor_scalar_max(den[:], acc[:, F_IN:264], 1e-30)
                rec = wk.tile([P, H1], f32, tag="rec")
                nc.vector.reciprocal(rec[:], den[:])
                h = wk.tile([P, F_IN], f32, tag="h")
                nc.vector.tensor_tensor(
                    out=h[:].rearrange("p (d h) -> p d h", h=H1),
                    in0=acc[:, 0:F_IN].rearrange("p (d h) -> p d h", h=H1),
                    in1=rec[:, None, :].to_broadcast([P, HD, H1]),
                    op=Alu.mult)
                hb = wk.tile([P, F_IN], f32, tag="hb")
                nc.vector.tensor_tensor(out=hb[:], in0=h[:], in1=b1B[:],
                                        op=Alu.add)
                nc.scalar.activation(hb[:], hb[:], Act.Relu)

                f2 = ps_f2.tile([P, 42], f32, space="PSUM", tag="f2")
                for j in range(2):
                    ht_ps = ps_t.tile([P, 272], f32, space="PSUM", tag="htp")
                    nc.tensor.transpose(ht_ps[:, 0:P], hb[:, j * P:(j + 1) * P],
                                        ident[:])
                    ht = wk.tile([P, P], bf16, tag="ht")
                    nc.scalar.copy(out=ht[:], in_=ht_ps[:, 0:P])
                    nc.tensor.matmul(f2[:], lhsT=ht[:], rhs=Wc2[:, j, :],
                                     start=(j == 0), stop=(j == 1))
                t2r = wk.tile([P, 41], bf16, tag="t2r")
                nc.scalar.copy(out=t2r[:], in_=f2[:, 0:41])
                nc.vector.tensor_copy(out=er2_sb[:, b:b + 1], in_=f2[:, 41:42])
                sl = slice(b * P, (b + 1) * P)
                if b < ABLOCKS:
                    oap = T2_localA[sl, 0:41]
                else:
                    oap = T2_localB[(b - ABLOCKS) * P:(b - ABLOCKS + 1) * P, 0:41]
                nc.sync.dma_start(oap, t2r[:])
                ccol += nbi
                if b == ABLOCKS - 1:
                    allgather(T2_localA, T2A)
            allgather(T2_localB, T2B)

            # ---- layer 2 edge phase ----
            ccol = 0
            g2_pref = []
            for b in range(NPREF):
                Gp = gp.tile([P, NBMAX, F2R], bf16, tag="g2")
                emit_seg(Gp, T2A[:], 0, int(n_a[b]), int(offA[b]), F2R)
                g2_pref.append(Gp)
            for b in range(BLOCKS_PER_CORE):
                nbi = int(nb_tot[b])
                na, nbk = int(n_a[b]), int(n_b[b])
                if b < NPREF:
                    G2 = g2_pref[b]
                else:
                    G2 = gp.tile([P, NBMAX, F2R], bf16, tag="g2")
                    emit_seg(G2, T2A[:], 0, na, int(offA[b]), F2R)
                emit_seg(G2, T2B[:], na, nbk, int(offB[b]), F2R)

                ic = indp.tile([P, NBMAX, 2 * P], f8, tag="ind")
                nc.sync.dma_start(ic[:, 0:nbi, :],
                                    t_ind[:, ccol * 2 * P:(ccol + nbi) * 2 * P])

                ers = ps_er.tile([P, NBH], f32, space="PSUM", tag="ers")
                for c in range(nbi):
                    nc.tensor.matmul(ers[:, c:c + 1],
                                     lhsT=ic[:, c, P:2 * P],
                                     rhs=er2_sb[:, b:b + 1],
                                     start=True, stop=True)
                ee = wk.tile([P, NBMAX], f32, tag="ee2")
                nc.vector.tensor_tensor(
                    out=ee[:, 0:nbi],
                    in0=G2[:, 0:nbi, C2:C2 + 1].rearrange("p a b -> p (a b)"),
                    in1=ers[:, 0:nbi], op=Alu.add)
                nc.scalar.activation(ee[:, 0:nbi], ee[:, 0:nbi], Act.Prelu,
                                     alpha=alpha[:, :1])
                w2 = wk.tile([P, NBMAX], bf16, tag="w2")
                nc.scalar.activation(w2[:, 0:nbi], ee[:, 0:nbi], Act.Exp)

                rhs2 = wk.tile([P, NBMAX, 41], bf16, tag="rhs2a")
                nc.vector.tensor_tensor(
                    out=rhs2[:, 0:nbi, 0:C2],
                    in0=G2[:, 0:nbi, 0:C2],
                    in1=w2[:, 0:nbi, None].to_broadcast([P, nbi, C2]),
                    op=Alu.mult)
                nc.scalar.copy(out=rhs2[:, 0:nbi, C2:41],
                               in_=w2[:, 0:nbi, None])
                acc = ps_agg.tile([P, 264], f32, space="PSUM", tag="agg")
                for c in range(nbi):
                    nc.tensor.matmul(acc[:, 0:41], lhsT=ic[:, c, 0:P],
                                     rhs=rhs2[:, c, :],
                                     start=(c == 0), stop=(c == nbi - 1))

                den = wk.tile([P, 1], f32, tag="den2")
                nc.vector.tensor_scalar_max(den[:], acc[:, C2:41], 1e-30)
                rec = wk.tile([P, 1], f32, tag="rec2")
                nc.vector.reciprocal(rec[:], den[:])
                o = wk.tile([P, C2], f32, tag="o")
                nc.vector.tensor_tensor(
                    out=o[:], in0=acc[:, 0:C2],
                    in1=rec[:, :1].to_broadcast([P, C2]), op=Alu.mult)
                nc.vector.tensor_tensor(out=o[:], in0=o[:], in1=b2B[:],
                                        op=Alu.add)
                nc.sync.dma_start(t_out[b * P:(b + 1) * P, :], o[:])
                ccol += nbi

    nc.compile()
    return nc


def kernel(features, src, dst, W1, attn_l1, attn_r1, b1, W2, attn_l2,
           attn_r2, b2):
    from concourse import bass_utils

    features = np.asarray(features, np.float32)
    src = np.asarray(src)
    dst = np.asarray(dst)
    W1 = np.asarray(W1, np.float32)
    attn_l1 = np.asarray(attn_l1, np.float32)
    attn_r1 = np.asarray(attn_r1, np.float32)
    b1 = np.asarray(b1, np.float32)
    W2 = np.asarray(W2, np.float32)
    attn_l2 = np.asarray(attn_l2, np.float32)
    attn_r2 = np.asarray(attn_r2, np.float32)
    b2 = np.asarray(b2, np.float32)

    g = _prep_graph(src, dst)
    per_core, NBMAX, CTOT = _build_core_inputs(
        g, features, W1, attn_l1, attn_r1, W2, attn_l2, attn_r2, b1, b2)

    IDXCOLS = per_core[0]['idx'].shape[1]
    nc = _build_program(g, NBMAX, CTOT, IDXCOLS)

    in_maps = []
    for pc in per_core:
        in_maps.append({
            "xT": pc['xT'], "idx": pc['idx'], "indc": pc['indc'],
            "Wcat1": pc['Wcat1'], "Wcat2": pc['Wcat2'],
            "b1B": pc['b1B'], "b2B": pc['b2B'],
        })

    res = bass_utils.run_bass_kernel_spmd(
        nc, in_maps, core_ids=list(range(NCORES)),
        trace=bool(int(os.environ.get('KTRACE', '0'))))
    kernel.last_result = res

    out = np.zeros((N_NODES, C2), np.float32)
    for c in range(NCORES):
        oc = res.results[c]["out2"]
        for i in range(BLOCKS_PER_CORE):
            b = g['blocks_at'][c][i]
            lo = b * P
            hi = min(lo + P, N_NODES)
            if hi > lo:
                out[lo:hi] = oc[i * P: i * P + (hi - lo)]
    return out


kernel.last_result = None
